# revision 3
# baseline (speedup 1.0000x reference)
"""Trainium2 Bass kernel for disparity cost-volume construction.

Reference computation (B=2, C=32, H=80, W=240, D=64):
    out[:, :C,  d, :, w] = x[:, :, :, w]      if w >= d else 0
    out[:, C:,  d, :, w] = y[:, :, :, w - d]  if w >= d else 0
    out shape [B, 2C, D, H, W]

Strategy: shard H across 8 cores (10 rows each; the disparity shift only
touches W so no halo). The problem is pure memory: ~78.6 MB of output
per core vs ~1.2 MB of input. Per core, load the x/y shards into SBUF
once, then issue one SBUF->DRAM DMA per (half, disparity) that writes the
shifted data region directly; the zero triangle (w < d) is never written
and stays at the runtime's zero-initialized output buffer contents
(ExternalOutput buffers are pre-zeroed np.zeros on both the native
run_neff path and the PJRT donation path).

Layout: on-chip partition index p = (b, c, hb) with hb splitting the 10
local rows into 2 groups of 5 — full 128-partition DMA width. Per-core
DRAM outputs use the custom layout [128, D, 1200] (1200 = 5 rows * 240 w)
so each per-disparity DMA lowers to a 3-dim access pattern. The host
reassembles the canonical [B, 2C, D, H, W] view afterwards.
"""

from contextlib import ExitStack

import numpy as np

B, C, H, W, D = 2, 32, 80, 240, 64
NCORES = 8
HL = H // NCORES  # local rows per core (10)
HB, H5 = 2, 5  # local rows split: 2 partition groups x 5 rows
P = B * C * HB  # 128 partitions
F = H5 * W  # 1200 free elements per (partition, d)

_CACHE: dict = {}


def _build():
    if "nc" in _CACHE:
        return _CACHE["nc"]

    import concourse.bacc as bacc
    import concourse.mybir as mybir
    import concourse.tile as tile

    f32 = mybir.dt.float32
    nc = bacc.Bacc("TRN2", target_bir_lowering=False, debug=False)

    x_t = nc.dram_tensor("x", [P, F], f32, kind="ExternalInput")
    y_t = nc.dram_tensor("y", [P, F], f32, kind="ExternalInput")
    ol_t = nc.dram_tensor("out_l", [P, D, F], f32, kind="ExternalOutput")
    or_t = nc.dram_tensor("out_r", [P, D, F], f32, kind="ExternalOutput")

    K_CH = 4  # rotating persistent left tiles (chains over d mod K_CH)

    with tile.TileContext(nc) as tc, ExitStack() as ctx:
        inpool = ctx.enter_context(tc.tile_pool(name="inp", bufs=1))
        chpool = ctx.enter_context(tc.tile_pool(name="ch", bufs=1))

        x_sb = inpool.tile([P, F], f32)
        y_sb = inpool.tile([P, F], f32)
        nc.sync.dma_start(x_sb, x_t.ap())
        nc.sync.dma_start(y_sb, y_t.ap())
        xv = x_sb.rearrange("p (h w) -> p h w", h=H5)
        yv = y_sb.rearrange("p (h w) -> p h w", h=H5)

        olv = ol_t.ap().rearrange("p d (h w) -> p d h w", h=H5)
        orv = or_t.ap().rearrange("p d (h w) -> p d h w", h=H5)

        # Left half: the row for disparity d+K_CH equals the row for d with
        # K_CH more leading columns zeroed. Keep K_CH persistent masked
        # copies of x and zero a 4-column strip per step instead of
        # recopying — left DMAs then write full 4800 B-run rows.
        ch = []
        for k in range(K_CH):
            t = chpool.tile([P, F], f32, tag=f"ch{k}")
            tv = t.rearrange("p (h w) -> p h w", h=H5)
            eng = nc.vector if k % 2 == 0 else nc.gpsimd
            if k > 0:
                eng.memset(tv[:, :, 0:k], 0.0)
            eng.tensor_copy(tv[:, :, k:W], xv[:, :, k:W])
            ch.append((t, tv))

        # Right half: shifted rows can't be updated in place; DMA the data
        # region straight out of y_sb (zero triangle stays at the output
        # buffer's pre-zeroed contents).
        for d in range(D):
            t, tv = ch[d % K_CH]
            nc.sync.dma_start(olv[:, d, :, :], t)
            if d + K_CH < D:
                eng = nc.vector if d % 2 == 0 else nc.gpsimd
                eng.memset(tv[:, :, d : d + K_CH], 0.0)
            nc.sync.dma_start(orv[:, d, :, d:W], yv[:, :, 0 : W - d])

    nc.compile()
    _CACHE["nc"] = nc
    return nc


def _shard_inputs(x: np.ndarray, y: np.ndarray):
    x = np.asarray(x, dtype=np.float32)
    y = np.asarray(y, dtype=np.float32)
    in_maps = []
    for k in range(NCORES):
        xs = np.ascontiguousarray(x[:, :, k * HL : (k + 1) * HL, :]).reshape(P, F)
        ys = np.ascontiguousarray(y[:, :, k * HL : (k + 1) * HL, :]).reshape(P, F)
        in_maps.append({"x": xs, "y": ys})
    return in_maps


def _gather(results) -> np.ndarray:
    full = np.empty((B, 2 * C, D, H, W), dtype=np.float32)
    for k in range(NCORES):
        h0 = k * HL
        for name, c0 in (("out_l", 0), ("out_r", C)):
            shard = (
                results[k][name]
                .reshape(B, C, HB, D, H5, W)
                .transpose(0, 1, 3, 2, 4, 5)
                .reshape(B, C, D, HL, W)
            )
            full[:, c0 : c0 + C, :, h0 : h0 + HL, :] = shard
    return full


def _run(x: np.ndarray, y: np.ndarray, trace: bool = False):
    from concourse.bass_utils import run_bass_kernel_spmd

    nc = _build()
    in_maps = _shard_inputs(x, y)
    res = run_bass_kernel_spmd(
        nc, in_maps, core_ids=list(range(NCORES)), trace=trace
    )
    return _gather(res.results), res


def kernel(x: np.ndarray, y: np.ndarray) -> np.ndarray:
    out, _ = _run(x, y, trace=False)
    return out


# revision 5
# speedup vs baseline: 1.0264x; 1.0264x over previous
"""Trainium2 Bass kernel for disparity cost-volume construction.

Reference computation (B=2, C=32, H=80, W=240, D=64):
    out[:, :C,  d, :, w] = x[:, :, :, w]      if w >= d else 0
    out[:, C:,  d, :, w] = y[:, :, :, w - d]  if w >= d else 0
    out shape [B, 2C, D, H, W]

Strategy: shard H across 8 cores (10 rows each; the disparity shift only
touches W so no halo). The problem is pure memory: ~78.6 MB of output
per core vs ~1.2 MB of input. Per core, load the x/y shards into SBUF
once, then issue one SBUF->DRAM DMA per (half, disparity) that writes the
shifted data region directly; the zero triangle (w < d) is never written
and stays at the runtime's zero-initialized output buffer contents
(ExternalOutput buffers are pre-zeroed np.zeros on both the native
run_neff path and the PJRT donation path).

Layout: on-chip partition index p = (b, c, hb) with hb splitting the 10
local rows into 2 groups of 5 — full 128-partition DMA width. Per-core
DRAM outputs use the custom layout [128, D, 1200] (1200 = 5 rows * 240 w)
so each per-disparity DMA lowers to a 3-dim access pattern. The host
reassembles the canonical [B, 2C, D, H, W] view afterwards.
"""

from contextlib import ExitStack

import numpy as np

B, C, H, W, D = 2, 32, 80, 240, 64
NCORES = 8
HL = H // NCORES  # local rows per core (10)
HB, H5 = 2, 5  # local rows split: 2 partition groups x 5 rows
P = B * C * HB  # 128 partitions
F = H5 * W  # 1200 free elements per (partition, d)

_CACHE: dict = {}


def _build():
    if "nc" in _CACHE:
        return _CACHE["nc"]

    import concourse.bacc as bacc
    import concourse.mybir as mybir
    import concourse.tile as tile

    f32 = mybir.dt.float32
    nc = bacc.Bacc("TRN2", target_bir_lowering=False, debug=False)

    x_t = nc.dram_tensor("x", [P, F], f32, kind="ExternalInput")
    y_t = nc.dram_tensor("y", [P, F], f32, kind="ExternalInput")
    ol_t = nc.dram_tensor("out_l", [P, D, F], f32, kind="ExternalOutput")
    or_t = nc.dram_tensor("out_r", [P, D, F], f32, kind="ExternalOutput")

    K_CH = 8  # rotating persistent left tiles (chains over d mod K_CH)

    with tile.TileContext(nc) as tc, ExitStack() as ctx:
        inpool = ctx.enter_context(tc.tile_pool(name="inp", bufs=1))
        chpool = ctx.enter_context(tc.tile_pool(name="ch", bufs=1))

        x_sb = inpool.tile([P, F], f32)
        y_sb = inpool.tile([P, F], f32)
        nc.sync.dma_start(x_sb, x_t.ap())
        nc.sync.dma_start(y_sb, y_t.ap())
        xv = x_sb.rearrange("p (h w) -> p h w", h=H5)
        yv = y_sb.rearrange("p (h w) -> p h w", h=H5)

        olv = ol_t.ap().rearrange("p d (h w) -> p d h w", h=H5)
        orv = or_t.ap().rearrange("p d (h w) -> p d h w", h=H5)

        # Left half: the row for disparity d+K_CH equals the row for d with
        # K_CH more leading columns zeroed. Keep K_CH persistent masked
        # copies of x and zero a 4-column strip per step instead of
        # recopying — left DMAs then write full 4800 B-run rows.
        ch = []
        for k in range(K_CH):
            t = chpool.tile([P, F], f32, tag=f"ch{k}")
            tv = t.rearrange("p (h w) -> p h w", h=H5)
            eng = nc.vector if k % 2 == 0 else nc.gpsimd
            if k > 0:
                eng.memset(tv[:, :, 0:k], 0.0)
            eng.tensor_copy(tv[:, :, k:W], xv[:, :, k:W])
            ch.append((t, tv))

        # Right half: shifted rows can't be updated in place; DMA the data
        # region straight out of y_sb (zero triangle stays at the output
        # buffer's pre-zeroed contents).
        # Left on the SP HWDGE ring, right on the ACT ring: a left DMA
        # stalled on its chain's memset must not head-of-line-block ready
        # right DMAs.
        for d in range(D):
            t, tv = ch[d % K_CH]
            nc.sync.dma_start(olv[:, d, :, :], t)
            if d + K_CH < D:
                eng = nc.vector if d % 2 == 0 else nc.gpsimd
                eng.memset(tv[:, :, d : d + K_CH], 0.0)
            nc.scalar.dma_start(orv[:, d, :, d:W], yv[:, :, 0 : W - d])

    nc.compile()
    _CACHE["nc"] = nc
    return nc


def _shard_inputs(x: np.ndarray, y: np.ndarray):
    x = np.asarray(x, dtype=np.float32)
    y = np.asarray(y, dtype=np.float32)
    in_maps = []
    for k in range(NCORES):
        xs = np.ascontiguousarray(x[:, :, k * HL : (k + 1) * HL, :]).reshape(P, F)
        ys = np.ascontiguousarray(y[:, :, k * HL : (k + 1) * HL, :]).reshape(P, F)
        in_maps.append({"x": xs, "y": ys})
    return in_maps


def _gather(results) -> np.ndarray:
    full = np.empty((B, 2 * C, D, H, W), dtype=np.float32)
    for k in range(NCORES):
        h0 = k * HL
        for name, c0 in (("out_l", 0), ("out_r", C)):
            shard = (
                results[k][name]
                .reshape(B, C, HB, D, H5, W)
                .transpose(0, 1, 3, 2, 4, 5)
                .reshape(B, C, D, HL, W)
            )
            full[:, c0 : c0 + C, :, h0 : h0 + HL, :] = shard
    return full


def _run(x: np.ndarray, y: np.ndarray, trace: bool = False):
    from concourse.bass_utils import run_bass_kernel_spmd

    nc = _build()
    in_maps = _shard_inputs(x, y)
    res = run_bass_kernel_spmd(
        nc, in_maps, core_ids=list(range(NCORES)), trace=trace
    )
    return _gather(res.results), res


def kernel(x: np.ndarray, y: np.ndarray) -> np.ndarray:
    out, _ = _run(x, y, trace=False)
    return out
